# revision 23
# baseline (speedup 1.0000x reference)
# Trainium2 Bass kernel for nn_BoltzmannMachine: sequential Gibbs sweep over
# N=8192 binary units.
#
# Algorithm (matches the jax reference bit-for-bit on binary states):
#   Work in permuted coordinates: unit a is updated at step a.
#   u <= sigmoid(x/T)  <=>  x >= T*logit(u) = thr  (T > 0), so the device
#   only compares against host-precomputed thresholds; no transcendentals.
#   x = x_base + L @ c with c the fire bits and L the strict lower triangle
#   of the permuted coupling matrix (columns scaled by the free mask).
#   Blocked at B=128: PE (TensorE) accumulates each block's x_base row in
#   PSUM out of 128-column matvec contributions (initial-state columns for
#   future blocks, updated columns u = r + f*c for past blocks), with the
#   fp32 weights split into a bf16 hi+lo pair so PE runs at bf16 rate with
#   ~2^-17 relative weight error (x error ~3e-6, far under the minimum
#   compare margin).
#
#   The sequential in-block sweep is the critical path: every DVE
#   instruction in the chain costs ~250ns of completion-sync overhead
#   (the next op's dispatch reads the z scalar via sequencer REG_PTR, so
#   it must wait for the previous op's SBUF write-ack). The baseline used
#   one fused op per unit (127 chain ops/block). Here a custom multi-uOp
#   DVE op (GIBBS_FUSED3) resolves THREE units per instruction: 4 init
#   elements resolve the 3 bits into swap flops (stages 5/6/7 + working
#   copies at 1/4) and write the two intra-chunk z updates; a 3-element
#   steady ping-pong then applies the rank-3 tail update
#   z' = (z + LA*b0) + (LB*b1 + LC*b2), combining products across elements
#   through the stage-7 ALU flop (1-cycle temporal feedback). 43 chain
#   ops/block instead of 127.
import numpy as np

import concourse.bass as bass  # noqa: F401
import concourse.mybir as mybir
from concourse import bacc, tile
from concourse import bass_utils
from concourse import dve_ops as _dve_ops
from concourse.dve_spec import Spec, Src0, Src1, C0, C1, Zero
from concourse.dve_uop import (
    ENABLE,
    AluInp,
    AluOp as UAluOp,
    DelayInp,
    DveOpSpec,
    InpSel,
    OutPath,
    OutSel,
    Trigger,
    UopConfig,
    UopDpConfig,
)

F32 = mybir.dt.float32
BF16 = mybir.dt.bfloat16
A = mybir.AluOpType

N_FULL = 8192
B = 128
K_FULL = N_FULL // B
N_CORES = 8


# --- GIBBS_AXPY: single-unit fused op (used for the last column) ----------- #


def _register_gibbs_axpy():
    """out = in0 + in1*(s0 >= 0). The (C0 + Src1*Zero) form keeps the compare
    stream-dependent so the lowering doesn't hoist it into a latch."""
    for op in _dve_ops.OPS:
        if op.name == "GIBBS_AXPY":
            return op
    op = _dve_ops.DveOp(
        "GIBBS_AXPY",
        Spec(
            body=Src0 + Src1 * ((C0 + Src1 * Zero) >= Zero),
            reference=lambda in0, in1, s0, s1, imm2: (
                in0 + in1 * (s0 >= 0.0)
            ).astype(np.float32),
        ),
        subdim=False,
        uops_sha={"v3": "4cebbc5d1fef964b", "v4": "54f17dbd90d668d1"},
    )
    _dve_ops.OPS.append(op)
    _dve_ops.CUSTOM_DVE_SPECS[op.name] = op.spec
    _dve_ops._SUB_OPCODE_FOR_NAME[op.name] = (
        max(_dve_ops._SUB_OPCODE_FOR_NAME.values()) + 1
    )
    return op


GIBBS_AXPY = _register_gibbs_axpy()


# --- GIBBS_FUSED3: three units per instruction (hand-built uOp FSM) -------- #
#
# Chunk = columns i, i+1, i+2 of the in-block strict-lower matrix.
#   b0 = (s0 >= 0)                              s0 = z[i]   (REG_PTR scalar)
#   S1 = s1 + L10*b0;  b1 = (S1 >= 0)           s1 = z[i+1] (REG_PTR scalar)
#   S2 = (z2 + L20*b0) + L21*b1;  b2 = (S2>=0)  z2 = in0[0]
#   out[0] = S1; out[1] = S2
#   tail j: out[2+t] = ((LA*b0) + z[j]) + ((LB*b1) + (LC*b2))
# src1 = [L10, L20, L21, then per j: (LC, LB, LA)]; src0 = [z2, tail z];
# out = [S1, S2, tail z'].

_P = AluInp.PREV_ALU_OUT
_CUR = AluInp.CURR_ALU_OUT
_SWP = AluInp.CURR_SWAP_OUT
_D0, _D1, _D2 = AluInp.PREV_DELAY_0, AluInp.PREV_DELAY_1, AluInp.PREV_DELAY_2


class _RelaxedUop(UopConfig):
    """Skip the delay-carried lint: FUSED3 parks a value in a delay-lane flop
    across elements (stage-4 lane-2), which the single-element lint rejects."""

    def validate(self, ver="v3"):
        pass


def _uop_base() -> _RelaxedUop:
    u = _RelaxedUop()
    for s in range(8):
        u.datapath_config[s] = UopDpConfig().pass_through_alu()
    return u


def _build_fused3_uops() -> list[UopConfig]:
    # 0: i1 — b0 = (C0 >= 0); latch @1 (init copy) and @5 (steady).
    i1 = _uop_base()
    i1.enable_input(InpSel.CONST_0, 1).enable_input(InpSel.ZERO, 2)
    i1.datapath_config[0].enable_alu(UAluOp.IS_GE, _D0, _D1)
    i1.datapath_config[1].swap_enable = ENABLE
    i1.datapath_config[5].swap_enable = ENABLE
    i1.repeat_count = 1
    i1.trigger = (Trigger.COUNT, Trigger.NONE, Trigger.NONE)
    i1.next_uop = (1, 0, 0)

    # 1: i2 — consume src1 (L10). S1 = C1 + L10*b0; b1 latch @4,@6; write S1.
    i2 = _uop_base()
    i2.enable_input(InpSel.CONST_1, 1)
    i2.enable_input(InpSel.SRC_1, 2)
    i2.enable_input(InpSel.ZERO, 3)
    i2.datapath_config[0].pass_through_delay(0, 1, 2)
    i2.datapath_config[1].enable_alu(UAluOp.MULTIPLY, _SWP, _D1).pass_through_delay(
        0, 2
    )
    i2.datapath_config[2].enable_alu(UAluOp.ADD, _P, _D0).pass_through_delay(2)
    i2.datapath_config[3].enable_alu(UAluOp.IS_GE, _P, _D2).enable_delay_from_src(
        DelayInp.PREV_ALU_OUT, 3
    )
    i2.datapath_config[4].pass_through_delay(3)
    i2.datapath_config[4].swap_enable = ENABLE
    i2.datapath_config[5].pass_through_delay(3)
    i2.datapath_config[6].pass_through_delay(3)
    i2.datapath_config[6].swap_enable = ENABLE
    i2.datapath_config[7].pass_through_delay(3)
    i2.enable_output(OutSel.DELAY_3, OutPath.WR0_LO)
    i2.require_inp1 = 1
    i2.repeat_count = 1
    i2.trigger = (Trigger.COUNT, Trigger.NONE, Trigger.NONE)
    i2.next_uop = (2, 0, 0)

    # 2: i3a — consume src0 (z2) + src1 (L20). P2 = z2 + L20*b0 parked at
    # lane-2 flops of stages 3 and 4.
    i3a = _uop_base()
    i3a.enable_input(InpSel.SRC_0, 1)
    i3a.enable_input(InpSel.SRC_1, 2)
    i3a.datapath_config[0].pass_through_delay(0, 1)
    i3a.datapath_config[1].enable_alu(UAluOp.MULTIPLY, _SWP, _D1).pass_through_delay(0)
    i3a.datapath_config[2].enable_alu(UAluOp.ADD, _P, _D0)
    i3a.datapath_config[3].enable_delay_from_src(DelayInp.PREV_ALU_OUT, 2)
    i3a.datapath_config[4].pass_through_delay(2)
    i3a.require_inp0 = 1
    i3a.require_inp1 = 1
    i3a.repeat_count = 1
    i3a.trigger = (Trigger.COUNT, Trigger.NONE, Trigger.NONE)
    i3a.next_uop = (3, 0, 0)

    # 3: i3b — consume src1 (L21). S2 = (L21*b1) + P2; b2 latch @7; write S2.
    # Lane 2 untouched through stage 4 so the stage-4 flop still holds P2.
    i3b = _uop_base()
    i3b.enable_input(InpSel.SRC_1, 1)
    i3b.enable_input(InpSel.ZERO, 2)
    for s in range(4):
        i3b.datapath_config[s].pass_through_delay(0)
    for s in range(6):
        i3b.datapath_config[s].pass_through_delay(1)
    i3b.datapath_config[4].enable_alu(UAluOp.MULTIPLY, _D0, _SWP)
    i3b.datapath_config[5].enable_alu(UAluOp.ADD, _P, _D2)
    i3b.datapath_config[6].enable_alu(UAluOp.IS_GE, _P, _D1).enable_delay_from_src(
        DelayInp.PREV_ALU_OUT, 3
    )
    i3b.datapath_config[7].pass_through_delay(3)
    i3b.datapath_config[7].swap_enable = ENABLE
    i3b.enable_output(OutSel.DELAY_3, OutPath.WR0_LO)
    i3b.require_inp1 = 1
    i3b.repeat_count = 1
    i3b.trigger = (Trigger.COUNT, Trigger.NONE, Trigger.NONE)
    i3b.next_uop = (4, 0, 0)

    # 4: sA — consume src1 (LC). t2 = LC * b2 left in stage-7 ALU flop.
    sA = _uop_base()
    sA.enable_input(InpSel.SRC_1, 0)
    sA.datapath_config[7].enable_alu(UAluOp.MULTIPLY, _P, _SWP)
    sA.require_inp1 = 1
    sA.repeat_count = 1
    sA.trigger = (Trigger.SRC_TENSOR_DONE, Trigger.COUNT, Trigger.NONE)
    sA.next_uop = (0, 5, 0)

    # 5: sB — consume src1 (LB). stage-7 flop <- (LB*b1) + t2.
    sB = _uop_base()
    sB.enable_input(InpSel.SRC_1, 0)
    sB.datapath_config[6].enable_alu(UAluOp.MULTIPLY, _P, _SWP)
    sB.datapath_config[7].enable_alu(UAluOp.ADD, _P, _CUR)
    sB.require_inp1 = 1
    sB.repeat_count = 1
    sB.trigger = (Trigger.SRC_TENSOR_DONE, Trigger.COUNT, Trigger.NONE)
    sB.next_uop = (0, 6, 0)

    # 6: sC — consume src0 (z) + src1 (LA). out = ((LA*b0) + z) + CURR.
    sC = _uop_base()
    sC.enable_input(InpSel.SRC_1, 0)
    sC.enable_input(InpSel.SRC_0, 1)
    for s in range(6):
        sC.datapath_config[s].pass_through_delay(0)
    sC.datapath_config[5].enable_alu(UAluOp.MULTIPLY, _P, _SWP)
    sC.datapath_config[6].enable_alu(UAluOp.ADD, _P, _D0)
    sC.datapath_config[7].enable_alu(UAluOp.ADD, _P, _CUR)
    sC.enable_output(OutSel.ALU_OUT, OutPath.WR0_LO)
    sC.require_inp0 = 1
    sC.require_inp1 = 1
    sC.repeat_count = 1
    sC.trigger = (Trigger.SRC_TENSOR_DONE, Trigger.COUNT, Trigger.NONE)
    sC.next_uop = (0, 4, 0)

    return [i1, i2, i3a, i3b, sA, sB, sC]


def _fused3_ref(in0, in1, s0, s1, imm2):
    f = np.float32
    z = np.asarray(in0, np.float32).reshape(-1)
    L = np.asarray(in1, np.float32).reshape(-1)
    s0 = f(np.asarray(s0).reshape(-1)[0])
    s1 = f(np.asarray(s1).reshape(-1)[0])
    b0 = f(1.0) if s0 >= 0 else f(0.0)
    S1 = f(s1 + f(L[0] * b0))
    b1 = f(1.0) if S1 >= 0 else f(0.0)
    S2 = f(f(z[0] + f(L[1] * b0)) + f(L[2] * b1))
    b2 = f(1.0) if S2 >= 0 else f(0.0)
    out = np.zeros(len(z) + 1, np.float32)
    out[0], out[1] = S1, S2
    for t in range(len(z) - 1):
        LC, LB, LA = L[3 + 3 * t], L[4 + 3 * t], L[5 + 3 * t]
        out[2 + t] = f(f(f(LA * b0) + z[1 + t]) + f(f(LB * b1) + f(LC * b2)))
    return out


class _RawDveOp:
    """DveOp-alike whose compile() returns hand-built uOps."""

    def __init__(self, name, uops, spec, subdim=False):
        self.name = name
        self.uops = uops
        self.spec = spec
        self.subdim = subdim
        self.perf_en = {}
        self._cache = {}

    def compile(self, ver):
        if ver not in self._cache:
            self._cache[ver] = DveOpSpec(
                name=self.name,
                opcode=_dve_ops.get_dve_sub_opcode(self.name),
                uops=self.uops,
                rd1_en=True,
            )
        return self._cache[ver]


def _register_fused3():
    for op in _dve_ops.OPS:
        if op.name == "GIBBS_FUSED3":
            return op
    op = _RawDveOp(
        "GIBBS_FUSED3",
        _build_fused3_uops(),
        Spec(
            body=Src0 + Src1 * ((C0 + C1) >= Zero),
            reference=lambda in0, in1, s0, s1, imm2: _fused3_ref(
                in0, in1, s0, s1, imm2
            ),
        ),
        subdim=False,
    )
    _dve_ops.OPS.append(op)
    _dve_ops.CUSTOM_DVE_SPECS[op.name] = op.spec
    _dve_ops._SUB_OPCODE_FOR_NAME[op.name] = (
        max(_dve_ops._SUB_OPCODE_FOR_NAME.values()) + 1
    )
    return op


GIBBS_FUSED3 = _register_fused3()


# --- GIBBS_MPHASE: m chunks (phases) per instruction ----------------------- #
#
# Phase FSM: entry -> j1 -> j2 -> i3a -> i3b -> (sA sB sC)*; sC's
# SUB_DIM_DONE (src0/src1/dst inner dims all wrap on the same element)
# chains back to j1 for the next phase. All z scalars come from the src0
# stream (no REG_PTR), so one instruction sweeps many chunks. Segments are
# padded to a common T0; the src0 read-port prefetches ~100 elements ahead
# of consumption, so phase segments must keep src0-seg = 3+T0 >= ~104
# (measured threshold) for the previous phase's writes to have committed.


def _build_mphase_uops():
    ent = _uop_base()
    for s in range(8):
        ent.datapath_config[s] = UopDpConfig().pass_through_alu()
    ent.repeat_count = 1
    ent.trigger = (Trigger.COUNT, Trigger.NONE, Trigger.NONE)
    ent.next_uop = (1, 0, 0)

    j1 = _uop_base()
    j1.enable_input(InpSel.SRC_0, 1).enable_input(InpSel.ZERO, 2)
    j1.datapath_config[0].enable_alu(UAluOp.IS_GE, _D0, _D1)
    j1.datapath_config[1].swap_enable = ENABLE
    j1.datapath_config[5].swap_enable = ENABLE
    j1.require_inp0 = 1
    j1.repeat_count = 1
    j1.trigger = (Trigger.COUNT, Trigger.NONE, Trigger.NONE)
    j1.next_uop = (2, 0, 0)

    j2 = _uop_base()
    j2.enable_input(InpSel.SRC_0, 1)
    j2.enable_input(InpSel.SRC_1, 2)
    j2.enable_input(InpSel.ZERO, 3)
    j2.datapath_config[0].pass_through_delay(0, 1, 2)
    j2.datapath_config[1].enable_alu(UAluOp.MULTIPLY, _SWP, _D1).pass_through_delay(
        0, 2
    )
    j2.datapath_config[2].enable_alu(UAluOp.ADD, _P, _D0).pass_through_delay(2)
    j2.datapath_config[3].enable_alu(UAluOp.IS_GE, _P, _D2).enable_delay_from_src(
        DelayInp.PREV_ALU_OUT, 3
    )
    j2.datapath_config[4].pass_through_delay(3)
    j2.datapath_config[4].swap_enable = ENABLE
    j2.datapath_config[5].pass_through_delay(3)
    j2.datapath_config[6].pass_through_delay(3)
    j2.datapath_config[6].swap_enable = ENABLE
    j2.datapath_config[7].pass_through_delay(3)
    j2.enable_output(OutSel.DELAY_3, OutPath.WR0_LO)
    j2.require_inp0 = 1
    j2.require_inp1 = 1
    j2.repeat_count = 1
    j2.trigger = (Trigger.COUNT, Trigger.NONE, Trigger.NONE)
    j2.next_uop = (3, 0, 0)

    i3a = _uop_base()
    i3a.enable_input(InpSel.SRC_0, 1)
    i3a.enable_input(InpSel.SRC_1, 2)
    i3a.datapath_config[0].pass_through_delay(0, 1)
    i3a.datapath_config[1].enable_alu(UAluOp.MULTIPLY, _SWP, _D1).pass_through_delay(0)
    i3a.datapath_config[2].enable_alu(UAluOp.ADD, _P, _D0)
    i3a.datapath_config[3].enable_delay_from_src(DelayInp.PREV_ALU_OUT, 2)
    i3a.datapath_config[4].pass_through_delay(2)
    i3a.require_inp0 = 1
    i3a.require_inp1 = 1
    i3a.repeat_count = 1
    i3a.trigger = (Trigger.COUNT, Trigger.NONE, Trigger.NONE)
    i3a.next_uop = (4, 0, 0)

    i3b = _uop_base()
    i3b.enable_input(InpSel.SRC_1, 1)
    i3b.enable_input(InpSel.ZERO, 2)
    for s in range(4):
        i3b.datapath_config[s].pass_through_delay(0)
    for s in range(6):
        i3b.datapath_config[s].pass_through_delay(1)
    i3b.datapath_config[4].enable_alu(UAluOp.MULTIPLY, _D0, _SWP)
    i3b.datapath_config[5].enable_alu(UAluOp.ADD, _P, _D2)
    i3b.datapath_config[6].enable_alu(UAluOp.IS_GE, _P, _D1).enable_delay_from_src(
        DelayInp.PREV_ALU_OUT, 3
    )
    i3b.datapath_config[7].pass_through_delay(3)
    i3b.datapath_config[7].swap_enable = ENABLE
    i3b.enable_output(OutSel.DELAY_3, OutPath.WR0_LO)
    i3b.require_inp1 = 1
    i3b.repeat_count = 1
    i3b.trigger = (Trigger.COUNT, Trigger.NONE, Trigger.NONE)
    i3b.next_uop = (5, 0, 0)

    sA = _uop_base()
    sA.enable_input(InpSel.SRC_1, 0)
    sA.datapath_config[7].enable_alu(UAluOp.MULTIPLY, _P, _SWP)
    sA.require_inp1 = 1
    sA.repeat_count = 1
    sA.trigger = (Trigger.SRC_TENSOR_DONE, Trigger.COUNT, Trigger.NONE)
    sA.next_uop = (0, 6, 0)

    sB = _uop_base()
    sB.enable_input(InpSel.SRC_1, 0)
    sB.datapath_config[6].enable_alu(UAluOp.MULTIPLY, _P, _SWP)
    sB.datapath_config[7].enable_alu(UAluOp.ADD, _P, _CUR)
    sB.require_inp1 = 1
    sB.repeat_count = 1
    sB.trigger = (Trigger.SRC_TENSOR_DONE, Trigger.COUNT, Trigger.NONE)
    sB.next_uop = (0, 7, 0)

    sC = _uop_base()
    sC.enable_input(InpSel.SRC_1, 0)
    sC.enable_input(InpSel.SRC_0, 1)
    for s in range(6):
        sC.datapath_config[s].pass_through_delay(0)
    sC.datapath_config[5].enable_alu(UAluOp.MULTIPLY, _P, _SWP)
    sC.datapath_config[6].enable_alu(UAluOp.ADD, _P, _D0)
    sC.datapath_config[7].enable_alu(UAluOp.ADD, _P, _CUR)
    sC.enable_output(OutSel.ALU_OUT, OutPath.WR0_LO)
    sC.require_inp0 = 1
    sC.require_inp1 = 1
    sC.repeat_count = 1
    sC.trigger = (Trigger.SRC_TENSOR_DONE, Trigger.SUB_DIM_DONE, Trigger.COUNT)
    sC.next_uop = (0, 1, 5)

    return [ent, j1, j2, i3a, i3b, sA, sB, sC]


def _register_mphase():
    for op in _dve_ops.OPS:
        if op.name == "GIBBS_MPHASE":
            return op
    op = _RawDveOp(
        "GIBBS_MPHASE",
        _build_mphase_uops(),
        Spec(body=Src0 + Src1 * (Src0 >= Zero), reference=None),
        subdim=True,
    )
    _dve_ops.OPS.append(op)
    _dve_ops.CUSTOM_DVE_SPECS[op.name] = op.spec
    _dve_ops._SUB_OPCODE_FOR_NAME[op.name] = (
        max(_dve_ops._SUB_OPCODE_FOR_NAME.values()) + 1
    )
    return op


GIBBS_MPHASE = _register_mphase()

# Sweep plan: chunks c = 0..42 at columns i = 3c, tail T = 125-3c.
# Merged groups (phase segments padded to T0, src0-seg = 3+T0 must beat the
# ~100-element read prefetch): [(i0, m, T0)].  Chunks with T < 26 pay less
# as completion-synced singles (FUSED3), then the last column as AXPY.
MP_GROUPS = [(0, 9, 125), (27, 25, 110)]
SINGLE_CHUNKS = [(i, 125 - i) for i in range(102, 126, 3)]  # (col, tail)
ZSCRATCH = 112  # z[108 + 112 + 2] = z[222] worst-case read/write overrun


# Per-block sweep stream layout:
#   [bias(0:B)] + merged-group segments + single-chunk streams + [axpy L].
def _stream_layout():
    pos = B
    glay = []
    for i0, m, T0 in MP_GROUPS:
        seg = 3 + 3 * T0
        glay.append((i0, m, T0, pos, seg))
        pos += m * seg
    slay = []
    for i, T in SINGLE_CHUNKS:
        slay.append((i, T, pos))
        pos += 3 + 3 * T
    return glay, slay, pos  # stream[pos] = axpy L; PACK = pos + 1


_GLAY, _SLAY, _AXPY_OFF = _stream_layout()
PACK = _AXPY_OFF + 1


def host_prep(w, initial_state, clamping_degree, T, perm, rand_u, N=N_FULL):
    K = N // B
    T = float(np.asarray(T))
    perm = np.asarray(perm).astype(np.int64)

    wp = np.asarray(w, dtype=np.float32)[perm][:, perm]
    s0p = np.asarray(initial_state, dtype=np.float32)[perm]
    f = (np.asarray(clamping_degree)[perm] == 0).astype(np.float32)
    r = s0p * (1.0 - f)
    uu = np.asarray(rand_u, dtype=np.float64)
    with np.errstate(divide="ignore"):
        thr = (T * (np.log(uu) - np.log1p(-uu))).astype(np.float32)

    WPT = np.ascontiguousarray(wp.T)

    # in-block base contributions (upper-incl-diag @ s0p + strict-lower @ r)
    xb = np.zeros(N, dtype=np.float32)
    for k in range(K):
        blk = slice(k * B, (k + 1) * B)
        Wb = wp[blk, blk]
        xb[blk] = (np.triu(Wb, 0) @ s0p[blk] + np.tril(Wb, -1) @ r[blk]).astype(
            np.float32
        )
    bias = (xb - thr).astype(np.float32)

    # wstrips[k][b, l*B+c] = WPT[l*B+b, k*B+c], split into bf16 hi + lo
    tmp = WPT.reshape(K, B, K, B)
    wstrips = np.ascontiguousarray(tmp.transpose(2, 1, 0, 3)).reshape(K, B, N)
    whi = wstrips.astype(mybir.dt.np(BF16))
    wlo = (wstrips - whi.astype(np.float32)).astype(mybir.dt.np(BF16))

    # sweep stream: bias + merged phase segments + singles + axpy coefficient.
    def pack_chunk(dst, s, ldT, i):
        """[L10, L20, L21, per tail j: (LC, LB, LA)] at dst[s:]; tail len."""
        T = B - 3 - i
        dst[s : s + 3] = (ldT[i, i + 1], ldT[i, i + 2], ldT[i + 1, i + 2])
        if T > 0:
            tri = np.stack(
                [ldT[i + 2, i + 3 :], ldT[i + 1, i + 3 :], ldT[i, i + 3 :]], axis=1
            )
            dst[s + 3 : s + 3 + 3 * T] = tri.ravel()

    sweep = np.zeros((K, PACK), dtype=np.float32)
    for k in range(K):
        blk = slice(k * B, (k + 1) * B)
        ldT = np.triu(WPT[blk, blk] * f[blk][:, None], 1)  # ldT[c,j]=L[j,c]
        sweep[k, :B] = bias[k * B : (k + 1) * B]
        for i0, m, T0, pos, seg in _GLAY:
            for p in range(m):
                pack_chunk(sweep[k], pos + p * seg, ldT, i0 + 3 * p)
        for i, T, pos in _SLAY:
            pack_chunk(sweep[k], pos, ldT, i)
        sweep[k, PACK - 1] = ldT[B - 2, B - 1]

    colsT = lambda v: np.ascontiguousarray(v.reshape(K, B).T)

    dev = {
        "whi": whi,
        "wlo": wlo,
        "sweep": sweep,
        "s0cols": colsT(s0p).astype(mybir.dt.np(BF16)),  # binary: exact
        "fcols": colsT(f),
        "rcols": colsT(r),
    }
    aux = {"perm": perm, "s0p": s0p, "f": f, "N": N}
    return dev, aux


def assemble_output(c_bits, aux):
    f, s0p, perm, N = aux["f"], aux["s0p"], aux["perm"], aux["N"]
    final_p = f * c_bits.astype(np.float32) + (1.0 - f) * s0p
    out = np.zeros(N, dtype=np.float32)
    out[perm] = final_p
    return out


def build(N=N_FULL, all_free=True):
    K = N // B
    nc = bacc.Bacc("TRN2", target_bir_lowering=False, debug=False)

    whi_d = nc.dram_tensor("whi", [K, B, N], BF16, kind="ExternalInput")
    wlo_d = nc.dram_tensor("wlo", [K, B, N], BF16, kind="ExternalInput")
    sweep_d = nc.dram_tensor("sweep", [K, PACK], F32, kind="ExternalInput")
    s0cols_d = nc.dram_tensor("s0cols", [B, K], BF16, kind="ExternalInput")
    fcols_d = nc.dram_tensor("fcols", [B, K], F32, kind="ExternalInput")
    rcols_d = nc.dram_tensor("rcols", [B, K], F32, kind="ExternalInput")
    out_d = nc.dram_tensor("c_out", [1, N], F32, kind="ExternalOutput")

    with tile.TileContext(nc) as tc:
        with (
            tc.tile_pool(name="resident", bufs=1) as res,
            tc.tile_pool(name="wpool", bufs=3) as wpool,
            tc.tile_pool(name="ldpool", bufs=2) as ldpool,
            tc.tile_pool(name="zpool", bufs=3) as zpool,
            tc.tile_pool(name="accp", bufs=4, space="PSUM") as accp,
            tc.tile_pool(name="cpsum", bufs=3, space="PSUM") as cpsum,
        ):
            s0_sb = res.tile([B, K], BF16, tag="s0")
            nc.sync.dma_start(out=s0_sb[:, :], in_=s0cols_d.ap())
            f_sb = res.tile([B, K], F32, tag="f")
            nc.sync.dma_start(out=f_sb[:, :], in_=fcols_d.ap())
            r_sb = res.tile([B, K], F32, tag="r")
            nc.sync.dma_start(out=r_sb[:, :], in_=rcols_d.ap())
            u_sb = res.tile([B, K], BF16, tag="u")
            ones_sb = res.tile([1, 1], BF16, tag="ones")
            nc.vector.memset(ones_sb[:, :], 1.0)
            onesf_sb = res.tile([1, 1], F32, tag="onesf")
            nc.vector.memset(onesf_sb[:, :], 1.0)

            # Software-pipelined emission: block k+1's independent matmuls
            # (s0-side l>k+1 and u-side l<k) are emitted BEFORE block k's
            # sweep so the PE queue runs them DURING the sweep; only the
            # 2-matmul JIT (l=k, needs block k's bits) sits at the boundary.
            def load_block(k):
                # quarter-split DMAs spread across HW queues so each block's
                # weight traffic completes well within a sweep (both on HW and
                # in the scheduler's simulation — a monolithic 2MB descriptor
                # makes the sim think the kernel is DMA-bound and it then
                # queues the bits-dependent transpose ahead of the next
                # block's matmul burst, serializing PE behind the sweep).
                # sweep stream FIRST so it rides ahead of this block's 4MB of
                # weight traffic on the DMA rings — group 2 starts consuming
                # its upper half ~4us into the block's sweep.
                ldk = ldpool.tile([1, PACK], F32, tag="ldk")
                cut = _GLAY[1][3]  # start of group 2 in the stream
                nc.sync.dma_start(
                    out=ldk[:, 0:cut], in_=sweep_d.ap()[k : k + 1, 0:cut]
                )
                nc.sync.dma_start(
                    out=ldk[:, cut:PACK], in_=sweep_d.ap()[k : k + 1, cut:PACK]
                )
                whik = wpool.tile([B, N], BF16, tag="whik")
                wlok = wpool.tile([B, N], BF16, tag="wlok")
                q = N // 4
                for t, d in ((whik, whi_d), (wlok, wlo_d)):
                    for qi in range(4):
                        nc.sync.dma_start(
                            out=t[:, qi * q : (qi + 1) * q],
                            in_=d.ap()[k][:, qi * q : (qi + 1) * q],
                        )
                return whik, wlok, ldk

            def emit_burst(k, whik, wlok, close):
                """acc tile + all matmuls for block k except the JIT (l=k-1).
                close=True (block 0 only, no JIT) puts stop on the last one."""
                acc = accp.tile([1, B], F32, tag="acc")
                order = [l for l in range(k + 1, K)] + [
                    l for l in range(0, max(k - 1, 0))
                ]
                for idx, l in enumerate(order):
                    v = s0_sb if l > k else u_sb
                    for half, wt in ((0, whik), (1, wlok)):
                        nc.tensor.matmul(
                            acc[:, :],
                            v[:, l : l + 1],
                            wt[:, l * B : (l + 1) * B],
                            start=(idx == 0 and half == 0),
                            stop=(close and idx == len(order) - 1 and half == 1),
                        )
                return acc

            cur = load_block(0)
            acc = emit_burst(0, cur[0], cur[1], close=True)
            for k in range(K):
                whik, wlok, ldk = cur
                if k + 1 < K:
                    nxt = load_block(k + 1)
                    acc_next = emit_burst(k + 1, nxt[0], nxt[1], close=False)
                else:
                    nxt = acc_next = None

                # seed z = acc + bias (row layout, partition 0)
                z = zpool.tile([1, B + ZSCRATCH], F32, tag="z")
                nc.vector.tensor_tensor(
                    out=z[:, 0:B], in0=acc[:, :], in1=ldk[:, 0:B], op=A.add
                )

                def seg_ap(t, base, m, stride, seglen):
                    c = t[:, base : base + seglen].unsqueeze(1)
                    c.ap[1] = [stride, m]
                    return c

                # sweep: merged phase groups, then short singles, then axpy
                for i0, m, T0, pos, seg in _GLAY:
                    nc.vector._custom_dve(
                        GIBBS_MPHASE,
                        out=seg_ap(z, i0 + 1, m, 3, 2 + T0),
                        in0=seg_ap(z, i0, m, 3, 3 + T0),
                        in1=seg_ap(ldk, pos, m, seg, seg),
                    )
                for i, T, pos in _SLAY:
                    nc.vector._custom_dve(
                        GIBBS_FUSED3,
                        out=z[:, i + 1 : B],
                        in0=z[:, i + 2 : B],
                        in1=ldk[:, pos : pos + 3 + 3 * T],
                        s0=z[:, i : i + 1],
                        s1=z[:, i + 1 : i + 2],
                    )
                nc.vector._custom_dve(
                    GIBBS_AXPY,
                    out=z[:, B - 1 : B],
                    in0=z[:, B - 1 : B],
                    in1=ldk[:, PACK - 1 :],
                    s0=z[:, B - 2 : B - 1],
                )

                if k < K - 1:
                    # Scheduler-only fence: block k+1's matmul burst (emitted
                    # above) must be placed before the z-dependent
                    # transpose/JIT in the PE queue, so it overlaps the sweep.
                    tc.no_sync_barrier()
                    # Critical boundary chain: transpose z directly (PE),
                    # derive the u column with one is_ge (clamping_degree is
                    # all-zero per the harness spec, so u = bits), then the
                    # JIT matmuls. The bf16 bits extract for the output DMA
                    # happens off this chain, below.
                    cp = cpsum.tile([B, 1], F32, tag="cp")
                    if all_free:
                        nc.tensor.matmul(
                            cp[:, :], z[:, 0:B], onesf_sb[:, :], start=True, stop=True
                        )
                        nc.vector.tensor_scalar(
                            out=u_sb[:, k : k + 1],
                            in0=cp[:, :],
                            scalar1=0.0,
                            scalar2=None,
                            op0=A.is_ge,
                        )
                    else:
                        cbf0 = zpool.tile([1, B], BF16, tag="cbf0")
                        nc.vector.tensor_scalar(
                            out=cbf0[:, :],
                            in0=z[:, 0:B],
                            scalar1=0.0,
                            scalar2=None,
                            op0=A.is_ge,
                        )
                        nc.tensor.matmul(
                            cp[:, :], cbf0[:, :], ones_sb[:, :], start=True, stop=True
                        )
                        nc.vector.scalar_tensor_tensor(
                            out=u_sb[:, k : k + 1],
                            in0=cp[:, :],
                            scalar=f_sb[:, k : k + 1],
                            in1=r_sb[:, k : k + 1],
                            op0=A.mult,
                            op1=A.add,
                        )
                    # JIT: l=k contribution into block k+1's accumulator,
                    # closing its PSUM accumulation group.
                    for half, wt in ((0, nxt[0]), (1, nxt[1])):
                        nc.tensor.matmul(
                            acc_next[:, :],
                            u_sb[:, k : k + 1],
                            wt[:, k * B : (k + 1) * B],
                            start=False,
                            stop=(half == 1),
                        )
                # bits row (bf16 — bits are exact) for the output DMA
                cbf = zpool.tile([1, B], BF16, tag="cbf")
                nc.vector.tensor_scalar(
                    out=cbf[:, :],
                    in0=z[:, 0:B],
                    scalar1=0.0,
                    scalar2=None,
                    op0=A.is_ge,
                )
                nc.gpsimd.dma_start(
                    out=out_d.ap()[0:1, k * B : (k + 1) * B], in_=cbf[:, :]
                )
                if k < K - 1:
                    cur, acc = nxt, acc_next

    nc.compile()
    return nc


_NC_CACHE = {}


def _get_nc(N=N_FULL, all_free=True):
    key = (N, all_free)
    if key not in _NC_CACHE:
        _NC_CACHE[key] = build(N, all_free)
    return _NC_CACHE[key]


def kernel(w, initial_state, clamping_degree, T, perm, rand_u, _trace=False):
    dev, aux = host_prep(w, initial_state, clamping_degree, T, perm, rand_u)
    nc = _get_nc(N_FULL, bool(aux["f"].all()))
    res = bass_utils.run_bass_kernel_spmd(
        nc,
        [dict(dev) for _ in range(N_CORES)],
        core_ids=list(range(N_CORES)),
        trace=_trace,
    )
    c_bits = np.asarray(res.results[0]["c_out"]).reshape(-1)
    if _trace:
        kernel.last_exec_time_ns = res.exec_time_ns
        kernel.last_results = res
    return assemble_output(c_bits, aux).astype(np.asarray(initial_state).dtype)


# revision 24
# speedup vs baseline: 1.2110x; 1.2110x over previous
# Trainium2 Bass kernel for nn_BoltzmannMachine: sequential Gibbs sweep over
# N=8192 binary units.
#
# Algorithm (matches the jax reference bit-for-bit on binary states):
#   Work in permuted coordinates: unit a is updated at step a.
#   u <= sigmoid(x/T)  <=>  x >= T*logit(u) = thr  (T > 0), so the device
#   only compares against host-precomputed thresholds; no transcendentals.
#   x = x_base + L @ c with c the fire bits and L the strict lower triangle
#   of the permuted coupling matrix (columns scaled by the free mask).
#   Blocked at B=128: PE (TensorE) accumulates each block's x_base row in
#   PSUM out of 128-column matvec contributions (initial-state columns for
#   future blocks, updated columns u = r + f*c for past blocks), with the
#   fp32 weights split into a bf16 hi+lo pair so PE runs at bf16 rate with
#   ~2^-17 relative weight error (x error ~3e-6, far under the minimum
#   compare margin).
#
#   The sequential in-block sweep is the critical path: every DVE
#   instruction in the chain costs ~250ns of completion-sync overhead
#   (the next op's dispatch reads the z scalar via sequencer REG_PTR, so
#   it must wait for the previous op's SBUF write-ack). The baseline used
#   one fused op per unit (127 chain ops/block). Here a custom multi-uOp
#   DVE op (GIBBS_FUSED3) resolves THREE units per instruction: 4 init
#   elements resolve the 3 bits into swap flops (stages 5/6/7 + working
#   copies at 1/4) and write the two intra-chunk z updates; a 3-element
#   steady ping-pong then applies the rank-3 tail update
#   z' = (z + LA*b0) + (LB*b1 + LC*b2), combining products across elements
#   through the stage-7 ALU flop (1-cycle temporal feedback). 43 chain
#   ops/block instead of 127.
import numpy as np

import concourse.bass as bass  # noqa: F401
import concourse.mybir as mybir
from concourse import bacc, tile
from concourse import bass_utils
from concourse import dve_ops as _dve_ops
from concourse.dve_spec import Spec, Src0, Src1, C0, C1, Zero
from concourse.dve_uop import (
    ENABLE,
    AluInp,
    AluOp as UAluOp,
    DelayInp,
    DveOpSpec,
    InpSel,
    OutPath,
    OutSel,
    Trigger,
    UopConfig,
    UopDpConfig,
)

F32 = mybir.dt.float32
BF16 = mybir.dt.bfloat16
A = mybir.AluOpType

N_FULL = 8192
B = 128
K_FULL = N_FULL // B
N_CORES = 8


# --- GIBBS_AXPY: single-unit fused op (used for the last column) ----------- #


def _register_gibbs_axpy():
    """out = in0 + in1*(s0 >= 0). The (C0 + Src1*Zero) form keeps the compare
    stream-dependent so the lowering doesn't hoist it into a latch."""
    for op in _dve_ops.OPS:
        if op.name == "GIBBS_AXPY":
            return op
    op = _dve_ops.DveOp(
        "GIBBS_AXPY",
        Spec(
            body=Src0 + Src1 * ((C0 + Src1 * Zero) >= Zero),
            reference=lambda in0, in1, s0, s1, imm2: (
                in0 + in1 * (s0 >= 0.0)
            ).astype(np.float32),
        ),
        subdim=False,
        uops_sha={"v3": "4cebbc5d1fef964b", "v4": "54f17dbd90d668d1"},
    )
    _dve_ops.OPS.append(op)
    _dve_ops.CUSTOM_DVE_SPECS[op.name] = op.spec
    _dve_ops._SUB_OPCODE_FOR_NAME[op.name] = (
        max(_dve_ops._SUB_OPCODE_FOR_NAME.values()) + 1
    )
    return op


GIBBS_AXPY = _register_gibbs_axpy()


# --- GIBBS_FUSED3: three units per instruction (hand-built uOp FSM) -------- #
#
# Chunk = columns i, i+1, i+2 of the in-block strict-lower matrix.
#   b0 = (s0 >= 0)                              s0 = z[i]   (REG_PTR scalar)
#   S1 = s1 + L10*b0;  b1 = (S1 >= 0)           s1 = z[i+1] (REG_PTR scalar)
#   S2 = (z2 + L20*b0) + L21*b1;  b2 = (S2>=0)  z2 = in0[0]
#   out[0] = S1; out[1] = S2
#   tail j: out[2+t] = ((LA*b0) + z[j]) + ((LB*b1) + (LC*b2))
# src1 = [L10, L20, L21, then per j: (LC, LB, LA)]; src0 = [z2, tail z];
# out = [S1, S2, tail z'].

_P = AluInp.PREV_ALU_OUT
_CUR = AluInp.CURR_ALU_OUT
_SWP = AluInp.CURR_SWAP_OUT
_D0, _D1, _D2 = AluInp.PREV_DELAY_0, AluInp.PREV_DELAY_1, AluInp.PREV_DELAY_2


class _RelaxedUop(UopConfig):
    """Skip the delay-carried lint: FUSED3 parks a value in a delay-lane flop
    across elements (stage-4 lane-2), which the single-element lint rejects."""

    def validate(self, ver="v3"):
        pass


def _uop_base() -> _RelaxedUop:
    u = _RelaxedUop()
    for s in range(8):
        u.datapath_config[s] = UopDpConfig().pass_through_alu()
    return u


def _build_fused3_uops() -> list[UopConfig]:
    # 0: i1 — b0 = (C0 >= 0); latch @1 (init copy) and @5 (steady).
    i1 = _uop_base()
    i1.enable_input(InpSel.CONST_0, 1).enable_input(InpSel.ZERO, 2)
    i1.datapath_config[0].enable_alu(UAluOp.IS_GE, _D0, _D1)
    i1.datapath_config[1].swap_enable = ENABLE
    i1.datapath_config[5].swap_enable = ENABLE
    i1.repeat_count = 1
    i1.trigger = (Trigger.COUNT, Trigger.NONE, Trigger.NONE)
    i1.next_uop = (1, 0, 0)

    # 1: i2 — consume src1 (L10). S1 = C1 + L10*b0; b1 latch @4,@6; write S1.
    i2 = _uop_base()
    i2.enable_input(InpSel.CONST_1, 1)
    i2.enable_input(InpSel.SRC_1, 2)
    i2.enable_input(InpSel.ZERO, 3)
    i2.datapath_config[0].pass_through_delay(0, 1, 2)
    i2.datapath_config[1].enable_alu(UAluOp.MULTIPLY, _SWP, _D1).pass_through_delay(
        0, 2
    )
    i2.datapath_config[2].enable_alu(UAluOp.ADD, _P, _D0).pass_through_delay(2)
    i2.datapath_config[3].enable_alu(UAluOp.IS_GE, _P, _D2).enable_delay_from_src(
        DelayInp.PREV_ALU_OUT, 3
    )
    i2.datapath_config[4].pass_through_delay(3)
    i2.datapath_config[4].swap_enable = ENABLE
    i2.datapath_config[5].pass_through_delay(3)
    i2.datapath_config[6].pass_through_delay(3)
    i2.datapath_config[6].swap_enable = ENABLE
    i2.datapath_config[7].pass_through_delay(3)
    i2.enable_output(OutSel.DELAY_3, OutPath.WR0_LO)
    i2.require_inp1 = 1
    i2.repeat_count = 1
    i2.trigger = (Trigger.COUNT, Trigger.NONE, Trigger.NONE)
    i2.next_uop = (2, 0, 0)

    # 2: i3a — consume src0 (z2) + src1 (L20). P2 = z2 + L20*b0 parked at
    # lane-2 flops of stages 3 and 4.
    i3a = _uop_base()
    i3a.enable_input(InpSel.SRC_0, 1)
    i3a.enable_input(InpSel.SRC_1, 2)
    i3a.datapath_config[0].pass_through_delay(0, 1)
    i3a.datapath_config[1].enable_alu(UAluOp.MULTIPLY, _SWP, _D1).pass_through_delay(0)
    i3a.datapath_config[2].enable_alu(UAluOp.ADD, _P, _D0)
    i3a.datapath_config[3].enable_delay_from_src(DelayInp.PREV_ALU_OUT, 2)
    i3a.datapath_config[4].pass_through_delay(2)
    i3a.require_inp0 = 1
    i3a.require_inp1 = 1
    i3a.repeat_count = 1
    i3a.trigger = (Trigger.COUNT, Trigger.NONE, Trigger.NONE)
    i3a.next_uop = (3, 0, 0)

    # 3: i3b — consume src1 (L21). S2 = (L21*b1) + P2; b2 latch @7; write S2.
    # Lane 2 untouched through stage 4 so the stage-4 flop still holds P2.
    i3b = _uop_base()
    i3b.enable_input(InpSel.SRC_1, 1)
    i3b.enable_input(InpSel.ZERO, 2)
    for s in range(4):
        i3b.datapath_config[s].pass_through_delay(0)
    for s in range(6):
        i3b.datapath_config[s].pass_through_delay(1)
    i3b.datapath_config[4].enable_alu(UAluOp.MULTIPLY, _D0, _SWP)
    i3b.datapath_config[5].enable_alu(UAluOp.ADD, _P, _D2)
    i3b.datapath_config[6].enable_alu(UAluOp.IS_GE, _P, _D1).enable_delay_from_src(
        DelayInp.PREV_ALU_OUT, 3
    )
    i3b.datapath_config[7].pass_through_delay(3)
    i3b.datapath_config[7].swap_enable = ENABLE
    i3b.enable_output(OutSel.DELAY_3, OutPath.WR0_LO)
    i3b.require_inp1 = 1
    i3b.repeat_count = 1
    i3b.trigger = (Trigger.COUNT, Trigger.NONE, Trigger.NONE)
    i3b.next_uop = (4, 0, 0)

    # 4: sA — consume src1 (LC). t2 = LC * b2 left in stage-7 ALU flop.
    sA = _uop_base()
    sA.enable_input(InpSel.SRC_1, 0)
    sA.datapath_config[7].enable_alu(UAluOp.MULTIPLY, _P, _SWP)
    sA.require_inp1 = 1
    sA.repeat_count = 1
    sA.trigger = (Trigger.SRC_TENSOR_DONE, Trigger.COUNT, Trigger.NONE)
    sA.next_uop = (0, 5, 0)

    # 5: sB — consume src1 (LB). stage-7 flop <- (LB*b1) + t2.
    sB = _uop_base()
    sB.enable_input(InpSel.SRC_1, 0)
    sB.datapath_config[6].enable_alu(UAluOp.MULTIPLY, _P, _SWP)
    sB.datapath_config[7].enable_alu(UAluOp.ADD, _P, _CUR)
    sB.require_inp1 = 1
    sB.repeat_count = 1
    sB.trigger = (Trigger.SRC_TENSOR_DONE, Trigger.COUNT, Trigger.NONE)
    sB.next_uop = (0, 6, 0)

    # 6: sC — consume src0 (z) + src1 (LA). out = ((LA*b0) + z) + CURR.
    sC = _uop_base()
    sC.enable_input(InpSel.SRC_1, 0)
    sC.enable_input(InpSel.SRC_0, 1)
    for s in range(6):
        sC.datapath_config[s].pass_through_delay(0)
    sC.datapath_config[5].enable_alu(UAluOp.MULTIPLY, _P, _SWP)
    sC.datapath_config[6].enable_alu(UAluOp.ADD, _P, _D0)
    sC.datapath_config[7].enable_alu(UAluOp.ADD, _P, _CUR)
    sC.enable_output(OutSel.ALU_OUT, OutPath.WR0_LO)
    sC.require_inp0 = 1
    sC.require_inp1 = 1
    sC.repeat_count = 1
    sC.trigger = (Trigger.SRC_TENSOR_DONE, Trigger.COUNT, Trigger.NONE)
    sC.next_uop = (0, 4, 0)

    return [i1, i2, i3a, i3b, sA, sB, sC]


def _fused3_ref(in0, in1, s0, s1, imm2):
    f = np.float32
    z = np.asarray(in0, np.float32).reshape(-1)
    L = np.asarray(in1, np.float32).reshape(-1)
    s0 = f(np.asarray(s0).reshape(-1)[0])
    s1 = f(np.asarray(s1).reshape(-1)[0])
    b0 = f(1.0) if s0 >= 0 else f(0.0)
    S1 = f(s1 + f(L[0] * b0))
    b1 = f(1.0) if S1 >= 0 else f(0.0)
    S2 = f(f(z[0] + f(L[1] * b0)) + f(L[2] * b1))
    b2 = f(1.0) if S2 >= 0 else f(0.0)
    out = np.zeros(len(z) + 1, np.float32)
    out[0], out[1] = S1, S2
    for t in range(len(z) - 1):
        LC, LB, LA = L[3 + 3 * t], L[4 + 3 * t], L[5 + 3 * t]
        out[2 + t] = f(f(f(LA * b0) + z[1 + t]) + f(f(LB * b1) + f(LC * b2)))
    return out


class _RawDveOp:
    """DveOp-alike whose compile() returns hand-built uOps."""

    def __init__(self, name, uops, spec, subdim=False):
        self.name = name
        self.uops = uops
        self.spec = spec
        self.subdim = subdim
        self.perf_en = {}
        self._cache = {}

    def compile(self, ver):
        if ver not in self._cache:
            self._cache[ver] = DveOpSpec(
                name=self.name,
                opcode=_dve_ops.get_dve_sub_opcode(self.name),
                uops=self.uops,
                rd1_en=True,
            )
        return self._cache[ver]


def _register_fused3():
    for op in _dve_ops.OPS:
        if op.name == "GIBBS_FUSED3":
            return op
    op = _RawDveOp(
        "GIBBS_FUSED3",
        _build_fused3_uops(),
        Spec(
            body=Src0 + Src1 * ((C0 + C1) >= Zero),
            reference=lambda in0, in1, s0, s1, imm2: _fused3_ref(
                in0, in1, s0, s1, imm2
            ),
        ),
        subdim=False,
    )
    _dve_ops.OPS.append(op)
    _dve_ops.CUSTOM_DVE_SPECS[op.name] = op.spec
    _dve_ops._SUB_OPCODE_FOR_NAME[op.name] = (
        max(_dve_ops._SUB_OPCODE_FOR_NAME.values()) + 1
    )
    return op


GIBBS_FUSED3 = _register_fused3()


# --- GIBBS_MPHASE: m chunks (phases) per instruction ----------------------- #
#
# Phase FSM: entry -> j1 -> j2 -> i3a -> i3b -> (sA sB sC)*; sC's
# SUB_DIM_DONE (src0/src1/dst inner dims all wrap on the same element)
# chains back to j1 for the next phase. All z scalars come from the src0
# stream (no REG_PTR), so one instruction sweeps many chunks. Segments are
# padded to a common T0; the src0 read-port prefetches ~100 elements ahead
# of consumption, so phase segments must keep src0-seg = 3+T0 >= ~104
# (measured threshold) for the previous phase's writes to have committed.


def _build_mphase_uops():
    ent = _uop_base()
    for s in range(8):
        ent.datapath_config[s] = UopDpConfig().pass_through_alu()
    ent.repeat_count = 1
    ent.trigger = (Trigger.COUNT, Trigger.NONE, Trigger.NONE)
    ent.next_uop = (1, 0, 0)

    j1 = _uop_base()
    j1.enable_input(InpSel.SRC_0, 1).enable_input(InpSel.ZERO, 2)
    j1.datapath_config[0].enable_alu(UAluOp.IS_GE, _D0, _D1)
    j1.datapath_config[1].swap_enable = ENABLE
    j1.datapath_config[5].swap_enable = ENABLE
    j1.require_inp0 = 1
    j1.repeat_count = 1
    j1.trigger = (Trigger.COUNT, Trigger.NONE, Trigger.NONE)
    j1.next_uop = (2, 0, 0)

    j2 = _uop_base()
    j2.enable_input(InpSel.SRC_0, 1)
    j2.enable_input(InpSel.SRC_1, 2)
    j2.enable_input(InpSel.ZERO, 3)
    j2.datapath_config[0].pass_through_delay(0, 1, 2)
    j2.datapath_config[1].enable_alu(UAluOp.MULTIPLY, _SWP, _D1).pass_through_delay(
        0, 2
    )
    j2.datapath_config[2].enable_alu(UAluOp.ADD, _P, _D0).pass_through_delay(2)
    j2.datapath_config[3].enable_alu(UAluOp.IS_GE, _P, _D2).enable_delay_from_src(
        DelayInp.PREV_ALU_OUT, 3
    )
    j2.datapath_config[4].pass_through_delay(3)
    j2.datapath_config[4].swap_enable = ENABLE
    j2.datapath_config[5].pass_through_delay(3)
    j2.datapath_config[6].pass_through_delay(3)
    j2.datapath_config[6].swap_enable = ENABLE
    j2.datapath_config[7].pass_through_delay(3)
    j2.enable_output(OutSel.DELAY_3, OutPath.WR0_LO)
    j2.require_inp0 = 1
    j2.require_inp1 = 1
    j2.repeat_count = 1
    j2.trigger = (Trigger.COUNT, Trigger.NONE, Trigger.NONE)
    j2.next_uop = (3, 0, 0)

    i3a = _uop_base()
    i3a.enable_input(InpSel.SRC_0, 1)
    i3a.enable_input(InpSel.SRC_1, 2)
    i3a.datapath_config[0].pass_through_delay(0, 1)
    i3a.datapath_config[1].enable_alu(UAluOp.MULTIPLY, _SWP, _D1).pass_through_delay(0)
    i3a.datapath_config[2].enable_alu(UAluOp.ADD, _P, _D0)
    i3a.datapath_config[3].enable_delay_from_src(DelayInp.PREV_ALU_OUT, 2)
    i3a.datapath_config[4].pass_through_delay(2)
    i3a.require_inp0 = 1
    i3a.require_inp1 = 1
    i3a.repeat_count = 1
    i3a.trigger = (Trigger.COUNT, Trigger.NONE, Trigger.NONE)
    i3a.next_uop = (4, 0, 0)

    i3b = _uop_base()
    i3b.enable_input(InpSel.SRC_1, 1)
    i3b.enable_input(InpSel.ZERO, 2)
    for s in range(4):
        i3b.datapath_config[s].pass_through_delay(0)
    for s in range(6):
        i3b.datapath_config[s].pass_through_delay(1)
    i3b.datapath_config[4].enable_alu(UAluOp.MULTIPLY, _D0, _SWP)
    i3b.datapath_config[5].enable_alu(UAluOp.ADD, _P, _D2)
    i3b.datapath_config[6].enable_alu(UAluOp.IS_GE, _P, _D1).enable_delay_from_src(
        DelayInp.PREV_ALU_OUT, 3
    )
    i3b.datapath_config[7].pass_through_delay(3)
    i3b.datapath_config[7].swap_enable = ENABLE
    i3b.enable_output(OutSel.DELAY_3, OutPath.WR0_LO)
    i3b.require_inp1 = 1
    i3b.repeat_count = 1
    i3b.trigger = (Trigger.COUNT, Trigger.NONE, Trigger.NONE)
    i3b.next_uop = (5, 0, 0)

    sA = _uop_base()
    sA.enable_input(InpSel.SRC_1, 0)
    sA.datapath_config[7].enable_alu(UAluOp.MULTIPLY, _P, _SWP)
    sA.require_inp1 = 1
    sA.repeat_count = 1
    sA.trigger = (Trigger.SRC_TENSOR_DONE, Trigger.COUNT, Trigger.NONE)
    sA.next_uop = (0, 6, 0)

    sB = _uop_base()
    sB.enable_input(InpSel.SRC_1, 0)
    sB.datapath_config[6].enable_alu(UAluOp.MULTIPLY, _P, _SWP)
    sB.datapath_config[7].enable_alu(UAluOp.ADD, _P, _CUR)
    sB.require_inp1 = 1
    sB.repeat_count = 1
    sB.trigger = (Trigger.SRC_TENSOR_DONE, Trigger.COUNT, Trigger.NONE)
    sB.next_uop = (0, 7, 0)

    sC = _uop_base()
    sC.enable_input(InpSel.SRC_1, 0)
    sC.enable_input(InpSel.SRC_0, 1)
    for s in range(6):
        sC.datapath_config[s].pass_through_delay(0)
    sC.datapath_config[5].enable_alu(UAluOp.MULTIPLY, _P, _SWP)
    sC.datapath_config[6].enable_alu(UAluOp.ADD, _P, _D0)
    sC.datapath_config[7].enable_alu(UAluOp.ADD, _P, _CUR)
    sC.enable_output(OutSel.ALU_OUT, OutPath.WR0_LO)
    sC.require_inp0 = 1
    sC.require_inp1 = 1
    sC.repeat_count = 1
    sC.trigger = (Trigger.SRC_TENSOR_DONE, Trigger.SUB_DIM_DONE, Trigger.COUNT)
    sC.next_uop = (0, 1, 5)

    return [ent, j1, j2, i3a, i3b, sA, sB, sC]


def _register_mphase():
    for op in _dve_ops.OPS:
        if op.name == "GIBBS_MPHASE":
            return op
    op = _RawDveOp(
        "GIBBS_MPHASE",
        _build_mphase_uops(),
        Spec(body=Src0 + Src1 * (Src0 >= Zero), reference=None),
        subdim=True,
    )
    _dve_ops.OPS.append(op)
    _dve_ops.CUSTOM_DVE_SPECS[op.name] = op.spec
    _dve_ops._SUB_OPCODE_FOR_NAME[op.name] = (
        max(_dve_ops._SUB_OPCODE_FOR_NAME.values()) + 1
    )
    return op


GIBBS_MPHASE = _register_mphase()

# Sweep plan: chunks c = 0..42 at columns i = 3c, tail T = 125-3c.
# Merged groups (phase segments padded to T0, src0-seg = 3+T0 must beat the
# ~100-element read prefetch): [(i0, m, T0)].  Chunks with T < 26 pay less
# as completion-synced singles (FUSED3), then the last column as AXPY.
MP_GROUPS = [(0, 9, 125), (27, 25, 110)]
SINGLE_CHUNKS = [(i, 125 - i) for i in range(102, 126, 3)]  # (col, tail)
ZSCRATCH = 112  # z[108 + 112 + 2] = z[222] worst-case read/write overrun


# Per-block sweep stream layout:
#   [bias(0:B)] + merged-group segments + single-chunk streams + [axpy L].
def _stream_layout():
    pos = B
    glay = []
    for i0, m, T0 in MP_GROUPS:
        seg = 3 + 3 * T0
        glay.append((i0, m, T0, pos, seg))
        pos += m * seg
    slay = []
    for i, T in SINGLE_CHUNKS:
        slay.append((i, T, pos))
        pos += 3 + 3 * T
    return glay, slay, pos  # stream[pos] = axpy L; PACK = pos + 1


_GLAY, _SLAY, _AXPY_OFF = _stream_layout()
PACK = _AXPY_OFF + 1


def host_prep(w, initial_state, clamping_degree, T, perm, rand_u, N=N_FULL):
    K = N // B
    T = float(np.asarray(T))
    perm = np.asarray(perm).astype(np.int64)

    wp = np.asarray(w, dtype=np.float32)[perm][:, perm]
    s0p = np.asarray(initial_state, dtype=np.float32)[perm]
    f = (np.asarray(clamping_degree)[perm] == 0).astype(np.float32)
    r = s0p * (1.0 - f)
    uu = np.asarray(rand_u, dtype=np.float64)
    with np.errstate(divide="ignore"):
        thr = (T * (np.log(uu) - np.log1p(-uu))).astype(np.float32)

    WPT = np.ascontiguousarray(wp.T)

    # in-block base contributions (upper-incl-diag @ s0p + strict-lower @ r)
    xb = np.zeros(N, dtype=np.float32)
    for k in range(K):
        blk = slice(k * B, (k + 1) * B)
        Wb = wp[blk, blk]
        xb[blk] = (np.triu(Wb, 0) @ s0p[blk] + np.tril(Wb, -1) @ r[blk]).astype(
            np.float32
        )
    bias = (xb - thr).astype(np.float32)

    # wstrips[k][b, l*B+c] = WPT[l*B+b, k*B+c], split into bf16 hi + lo
    tmp = WPT.reshape(K, B, K, B)
    wstrips = np.ascontiguousarray(tmp.transpose(2, 1, 0, 3)).reshape(K, B, N)
    whi = wstrips.astype(mybir.dt.np(BF16))
    wlo = (wstrips - whi.astype(np.float32)).astype(mybir.dt.np(BF16))

    # sweep stream: bias + merged phase segments + singles + axpy coefficient.
    def pack_chunk(dst, s, ldT, i):
        """[L10, L20, L21, per tail j: (LC, LB, LA)] at dst[s:]; tail len."""
        T = B - 3 - i
        dst[s : s + 3] = (ldT[i, i + 1], ldT[i, i + 2], ldT[i + 1, i + 2])
        if T > 0:
            tri = np.stack(
                [ldT[i + 2, i + 3 :], ldT[i + 1, i + 3 :], ldT[i, i + 3 :]], axis=1
            )
            dst[s + 3 : s + 3 + 3 * T] = tri.ravel()

    sweep = np.zeros((K, PACK), dtype=np.float32)
    for k in range(K):
        blk = slice(k * B, (k + 1) * B)
        ldT = np.triu(WPT[blk, blk] * f[blk][:, None], 1)  # ldT[c,j]=L[j,c]
        sweep[k, :B] = bias[k * B : (k + 1) * B]
        for i0, m, T0, pos, seg in _GLAY:
            for p in range(m):
                pack_chunk(sweep[k], pos + p * seg, ldT, i0 + 3 * p)
        for i, T, pos in _SLAY:
            pack_chunk(sweep[k], pos, ldT, i)
        sweep[k, PACK - 1] = ldT[B - 2, B - 1]

    colsT = lambda v: np.ascontiguousarray(v.reshape(K, B).T)

    dev = {
        "whi": whi,
        "wlo": wlo,
        "sweep": sweep,
        "s0cols": colsT(s0p).astype(mybir.dt.np(BF16)),  # binary: exact
        "fcols": colsT(f),
        "rcols": colsT(r),
    }
    aux = {"perm": perm, "s0p": s0p, "f": f, "N": N}
    return dev, aux


def assemble_output(c_bits, aux):
    f, s0p, perm, N = aux["f"], aux["s0p"], aux["perm"], aux["N"]
    final_p = f * c_bits.astype(np.float32) + (1.0 - f) * s0p
    out = np.zeros(N, dtype=np.float32)
    out[perm] = final_p
    return out


def build(N=N_FULL, all_free=True):
    K = N // B
    nc = bacc.Bacc("TRN2", target_bir_lowering=False, debug=False)

    whi_d = nc.dram_tensor("whi", [K, B, N], BF16, kind="ExternalInput")
    wlo_d = nc.dram_tensor("wlo", [K, B, N], BF16, kind="ExternalInput")
    sweep_d = nc.dram_tensor("sweep", [K, PACK], F32, kind="ExternalInput")
    s0cols_d = nc.dram_tensor("s0cols", [B, K], BF16, kind="ExternalInput")
    fcols_d = nc.dram_tensor("fcols", [B, K], F32, kind="ExternalInput")
    rcols_d = nc.dram_tensor("rcols", [B, K], F32, kind="ExternalInput")
    out_d = nc.dram_tensor("c_out", [1, N], F32, kind="ExternalOutput")

    with tile.TileContext(nc) as tc:
        with (
            tc.tile_pool(name="resident", bufs=1) as res,
            tc.tile_pool(name="wpool", bufs=3) as wpool,
            tc.tile_pool(name="ldpool", bufs=2) as ldpool,
            tc.tile_pool(name="zpool", bufs=3) as zpool,
            tc.tile_pool(name="accp", bufs=4, space="PSUM") as accp,
            tc.tile_pool(name="cpsum", bufs=3, space="PSUM") as cpsum,
        ):
            s0_sb = res.tile([B, K], BF16, tag="s0")
            nc.sync.dma_start(out=s0_sb[:, :], in_=s0cols_d.ap())
            f_sb = res.tile([B, K], F32, tag="f")
            nc.sync.dma_start(out=f_sb[:, :], in_=fcols_d.ap())
            r_sb = res.tile([B, K], F32, tag="r")
            nc.sync.dma_start(out=r_sb[:, :], in_=rcols_d.ap())
            u_sb = res.tile([B, K], BF16, tag="u")
            ones_sb = res.tile([1, 1], BF16, tag="ones")
            nc.vector.memset(ones_sb[:, :], 1.0)
            onesf_sb = res.tile([1, 1], F32, tag="onesf")
            nc.vector.memset(onesf_sb[:, :], 1.0)

            # Software-pipelined emission: block k+1's independent matmuls
            # (s0-side l>k+1 and u-side l<k) are emitted BEFORE block k's
            # sweep so the PE queue runs them DURING the sweep; only the
            # 2-matmul JIT (l=k, needs block k's bits) sits at the boundary.
            def load_block(k):
                # quarter-split DMAs spread across HW queues so each block's
                # weight traffic completes well within a sweep (both on HW and
                # in the scheduler's simulation — a monolithic 2MB descriptor
                # makes the sim think the kernel is DMA-bound and it then
                # queues the bits-dependent transpose ahead of the next
                # block's matmul burst, serializing PE behind the sweep).
                # sweep stream on the SW-DGE (gpsimd) queues — separate from
                # the weight traffic on the HW rings; issued first so the
                # Pool engine starts its descriptors early.
                ldk = ldpool.tile([1, PACK], F32, tag="ldk")
                cut = _GLAY[1][3]  # start of group 2 in the stream
                nc.gpsimd.dma_start(
                    out=ldk[:, 0:cut], in_=sweep_d.ap()[k : k + 1, 0:cut]
                )
                nc.gpsimd.dma_start(
                    out=ldk[:, cut:PACK], in_=sweep_d.ap()[k : k + 1, cut:PACK]
                )
                whik = wpool.tile([B, N], BF16, tag="whik")
                wlok = wpool.tile([B, N], BF16, tag="wlok")
                q = N // 4
                for t, d in ((whik, whi_d), (wlok, wlo_d)):
                    for qi in range(4):
                        nc.sync.dma_start(
                            out=t[:, qi * q : (qi + 1) * q],
                            in_=d.ap()[k][:, qi * q : (qi + 1) * q],
                        )
                return whik, wlok, ldk

            def emit_burst(k, whik, wlok, close):
                """acc tile + all matmuls for block k except the JIT (l=k-1).
                close=True (block 0 only, no JIT) puts stop on the last one."""
                acc = accp.tile([1, B], F32, tag="acc")
                order = [l for l in range(k + 1, K)] + [
                    l for l in range(0, max(k - 1, 0))
                ]
                for idx, l in enumerate(order):
                    v = s0_sb if l > k else u_sb
                    for half, wt in ((0, whik), (1, wlok)):
                        nc.tensor.matmul(
                            acc[:, :],
                            v[:, l : l + 1],
                            wt[:, l * B : (l + 1) * B],
                            start=(idx == 0 and half == 0),
                            stop=(close and idx == len(order) - 1 and half == 1),
                        )
                return acc

            cur = load_block(0)
            acc = emit_burst(0, cur[0], cur[1], close=True)
            for k in range(K):
                whik, wlok, ldk = cur
                if k + 1 < K:
                    nxt = load_block(k + 1)
                    acc_next = emit_burst(k + 1, nxt[0], nxt[1], close=False)
                else:
                    nxt = acc_next = None

                # seed z = acc + bias (row layout, partition 0)
                z = zpool.tile([1, B + ZSCRATCH], F32, tag="z")
                nc.vector.tensor_tensor(
                    out=z[:, 0:B], in0=acc[:, :], in1=ldk[:, 0:B], op=A.add
                )

                def seg_ap(t, base, m, stride, seglen):
                    c = t[:, base : base + seglen].unsqueeze(1)
                    c.ap[1] = [stride, m]
                    return c

                # sweep: merged phase groups, then short singles, then axpy
                for i0, m, T0, pos, seg in _GLAY:
                    nc.vector._custom_dve(
                        GIBBS_MPHASE,
                        out=seg_ap(z, i0 + 1, m, 3, 2 + T0),
                        in0=seg_ap(z, i0, m, 3, 3 + T0),
                        in1=seg_ap(ldk, pos, m, seg, seg),
                    )
                for i, T, pos in _SLAY:
                    nc.vector._custom_dve(
                        GIBBS_FUSED3,
                        out=z[:, i + 1 : B],
                        in0=z[:, i + 2 : B],
                        in1=ldk[:, pos : pos + 3 + 3 * T],
                        s0=z[:, i : i + 1],
                        s1=z[:, i + 1 : i + 2],
                    )
                nc.vector._custom_dve(
                    GIBBS_AXPY,
                    out=z[:, B - 1 : B],
                    in0=z[:, B - 1 : B],
                    in1=ldk[:, PACK - 1 :],
                    s0=z[:, B - 2 : B - 1],
                )

                if k < K - 1:
                    # Scheduler-only fence: block k+1's matmul burst (emitted
                    # above) must be placed before the z-dependent
                    # transpose/JIT in the PE queue, so it overlaps the sweep.
                    tc.no_sync_barrier()
                    # Critical boundary chain: transpose z directly (PE),
                    # derive the u column with one is_ge (clamping_degree is
                    # all-zero per the harness spec, so u = bits), then the
                    # JIT matmuls. The bf16 bits extract for the output DMA
                    # happens off this chain, below.
                    cp = cpsum.tile([B, 1], F32, tag="cp")
                    if all_free:
                        nc.tensor.matmul(
                            cp[:, :], z[:, 0:B], onesf_sb[:, :], start=True, stop=True
                        )
                        nc.vector.tensor_scalar(
                            out=u_sb[:, k : k + 1],
                            in0=cp[:, :],
                            scalar1=0.0,
                            scalar2=None,
                            op0=A.is_ge,
                        )
                    else:
                        cbf0 = zpool.tile([1, B], BF16, tag="cbf0")
                        nc.vector.tensor_scalar(
                            out=cbf0[:, :],
                            in0=z[:, 0:B],
                            scalar1=0.0,
                            scalar2=None,
                            op0=A.is_ge,
                        )
                        nc.tensor.matmul(
                            cp[:, :], cbf0[:, :], ones_sb[:, :], start=True, stop=True
                        )
                        nc.vector.scalar_tensor_tensor(
                            out=u_sb[:, k : k + 1],
                            in0=cp[:, :],
                            scalar=f_sb[:, k : k + 1],
                            in1=r_sb[:, k : k + 1],
                            op0=A.mult,
                            op1=A.add,
                        )
                    # JIT: l=k contribution into block k+1's accumulator,
                    # closing its PSUM accumulation group.
                    for half, wt in ((0, nxt[0]), (1, nxt[1])):
                        nc.tensor.matmul(
                            acc_next[:, :],
                            u_sb[:, k : k + 1],
                            wt[:, k * B : (k + 1) * B],
                            start=False,
                            stop=(half == 1),
                        )
                # bits row (bf16 — bits are exact) for the output DMA
                cbf = zpool.tile([1, B], BF16, tag="cbf")
                nc.vector.tensor_scalar(
                    out=cbf[:, :],
                    in0=z[:, 0:B],
                    scalar1=0.0,
                    scalar2=None,
                    op0=A.is_ge,
                )
                nc.gpsimd.dma_start(
                    out=out_d.ap()[0:1, k * B : (k + 1) * B], in_=cbf[:, :]
                )
                if k < K - 1:
                    cur, acc = nxt, acc_next

    nc.compile()
    return nc


_NC_CACHE = {}


def _get_nc(N=N_FULL, all_free=True):
    key = (N, all_free)
    if key not in _NC_CACHE:
        _NC_CACHE[key] = build(N, all_free)
    return _NC_CACHE[key]


def kernel(w, initial_state, clamping_degree, T, perm, rand_u, _trace=False):
    dev, aux = host_prep(w, initial_state, clamping_degree, T, perm, rand_u)
    nc = _get_nc(N_FULL, bool(aux["f"].all()))
    res = bass_utils.run_bass_kernel_spmd(
        nc,
        [dict(dev) for _ in range(N_CORES)],
        core_ids=list(range(N_CORES)),
        trace=_trace,
    )
    c_bits = np.asarray(res.results[0]["c_out"]).reshape(-1)
    if _trace:
        kernel.last_exec_time_ns = res.exec_time_ns
        kernel.last_results = res
    return assemble_output(c_bits, aux).astype(np.asarray(initial_state).dtype)
